# revision 30
# baseline (speedup 1.0000x reference)
import numpy as np

N, C, O, H, S = 30000, 128, 128, 8, 16
D = O // H            # 16
OS = 16
EPS = 1e-5
N_CORES = 8
OFFSETS = [15000, 30000]
NC_CLOUD = 15000          # points per cloud
NP_CORE = N // N_CORES    # 3750 query points per core
CHUNK = 32                # query points per compute chunk
E_CHUNK = CHUNK * S       # 512 edges per chunk
GRP = 3                   # chunks per softmax group (PSUM offsets 0/32/64)
N_GRP = 40                # groups per core (120 chunks, 3840 query slots)
N_CHUNKS = N_GRP * GRP
QPAD = N_CHUNKS * CHUNK   # 3840
E_GRP = GRP * E_CHUNK     # 2048 edges gathered per group
TROW = 256                # fp16 slots per gather row: x(128)|p_hi(3)|p_lo(3)|pad
GPU_ = 1                  # softmax groups per gather unit
E_GATHER = GPU_ * E_GRP   # idxs per dma_gather
SQ_SCALE = 2.0 ** -14     # sqr1 = SQ_SCALE*cr^2 (keeps eps*vr matmul weights normal)


def _f32(a):
    return np.ascontiguousarray(np.asarray(a, dtype=np.float32))


def _ln_np(x, g, b):
    m = x.mean(-1, keepdims=True, dtype=np.float32)
    v = ((x - m) ** 2).mean(-1, keepdims=True, dtype=np.float32)
    return (x - m) / np.sqrt(v + EPS) * g + b


def _kernel_numpy(p, x, idx, Wq, bq, Wk, bk, Wv, bv, Wp1, bp1, gp, betap, Wp2,
                  bp2, gw1, betaw1, Ww1, bw1, gw2, betaw2, Ww2, bw2):
    xq = (x @ Wq + bq).reshape(N, H, D)
    xk = x @ Wk + bk
    xv = x @ Wv + bv
    out = np.empty((N, O), dtype=np.float32)
    CH = N // N_CORES
    for s0 in range(0, N, CH):
        s1 = min(s0 + CH, N)
        ii = idx[s0:s1]
        kg = xk[ii].reshape(-1, S, H, D)
        vg = xv[ii].reshape(-1, S, H, D)
        pr = p[ii] - p[s0:s1, None, :]
        t = _ln_np(pr @ Wp1 + bp1, gp, betap)
        pe = np.maximum(t, 0.0) @ Wp2 + bp2
        pe = pe.reshape(-1, S, H, D)
        r = kg + pe - xq[s0:s1, None]
        w = np.maximum(_ln_np(r, gw1, betaw1), 0.0) @ Ww1 + bw1
        w = np.maximum(_ln_np(w, gw2, betaw2), 0.0) @ Ww2 + bw2
        wm = w.mean(-1, dtype=np.float32)
        wm = wm - wm.max(axis=1, keepdims=True)
        e = np.exp(wm)
        wsm = e / e.sum(axis=1, keepdims=True)
        out[s0:s1] = ((vg + pe) * wsm[..., None]).sum(axis=1).reshape(s1 - s0, O)
    return out


def _build_bass():
    """SPMD Bass program: one NeuronCore handles 3750 query points.

    Table-free: per group of 3 chunks one dma_gather of x|p rows
    (512 B/edge) straight from a host-built DRAM array. Specialized to
    gamma=1/beta=0 LayerNorm affines, which lets relu factor through the
    positive LN scales so LN1's rsqrt cancels out of LN2 exactly:
        cwh = w~ * rsqrt(m~ + eps^2)   (the eps*vr term is negligible).
    The remaining two rsqrts run as Ln + Exp(-0.5*) pairs so every
    activation (ln, exp, relu, square, copy) lives in one ACT table.
    Softmax exp is batched 3 chunks per PSUM bank (partition offsets
    0/32/64); normalization and output biases are applied on the host.
    """
    import concourse.bass as bass
    import concourse.bacc as bacc
    import concourse.tile as tile
    from concourse import mybir

    f32 = mybir.dt.float32
    f16 = mybir.dt.float16
    bf16 = mybir.dt.bfloat16
    i16 = mybir.dt.int16
    AF = mybir.ActivationFunctionType
    ALU = mybir.AluOpType
    AX = mybir.AxisListType

    nc = bacc.Bacc("TRN2", target_bir_lowering=False, debug=False,
                   num_devices=N_CORES)

    gtab = nc.declare_dram_parameter("gtab", [NC_CLOUD, TROW], f16,
                                     isOutput=False)
    idxw = nc.declare_dram_parameter("idxw", [128, (N_GRP // GPU_) * (E_GATHER // 16)], i16,
                                     isOutput=False)
    xqcw = nc.declare_dram_parameter("xqcw", [128, QPAD], f16, isOutput=False)
    pwcw = nc.declare_dram_parameter("pwcw", [4, QPAD], f32, isOutput=False)
    WkCm = nc.declare_dram_parameter("WkCm", [128, 128], f16, isOutput=False)
    Wvm = nc.declare_dram_parameter("Wvm", [128, 128], f16, isOutput=False)
    Wp2Cm = nc.declare_dram_parameter("Wp2Cm", [4, 128], f16, isOutput=False)
    Wp2m = nc.declare_dram_parameter("Wp2m", [4, 128], f16, isOutput=False)
    L6m = nc.declare_dram_parameter("L6m", [8, 4], f16, isOutput=False)
    M3m = nc.declare_dram_parameter("M3m", [4, 4], f16, isOutput=False)
    M1m = nc.declare_dram_parameter("M1m", [128, 128], f16, isOutput=False)
    W1cm = nc.declare_dram_parameter("W1cm", [128, 128], f16, isOutput=False)
    Wm32m = nc.declare_dram_parameter("Wm32m", [128, 32], f16, isOutput=False)
    O8m = nc.declare_dram_parameter("O8m", [96, 128], bf16, isOutput=False)
    # [128, 4] f32 constants: 0: EPS  1: EPS^2  2: -0.5
    cvec = nc.declare_dram_parameter("cvec", [128, 4], f32, isOutput=False)

    outT = nc.declare_dram_parameter("outT", [128, QPAD], f32, isOutput=True)
    s8T = nc.declare_dram_parameter("s8T", [128, N_GRP * 32], f32,
                                    isOutput=True)

    with tile.TileContext(nc) as tc, \
         nc.allow_low_precision(reason="fp16 intermediates are intentional"):
        with tc.tile_pool(name="persist", bufs=1) as pp:
            idx_sb = pp.tile([128, (N_GRP // GPU_) * (E_GATHER // 16)], i16)
            xqc_sb = pp.tile([128, QPAD], f16)
            pwc_sb = pp.tile([4, QPAD], f32)
            wkc_s = pp.tile([128, 128], f16)
            wv_s = pp.tile([128, 128], f16)
            wp2c_s = pp.tile([4, 128], f16)
            wp2_s = pp.tile([4, 128], f16)
            l6_s = pp.tile([8, 4], f16)
            m3_s = pp.tile([4, 4], f16)
            m1_s = pp.tile([128, 128], f16)
            w1c_s = pp.tile([128, 128], f16)
            wm32_s = pp.tile([128, 32], f16)
            o8_s = pp.tile([96, 128], bf16)
            cv = pp.tile([128, 4], f32)
            out_sb = pp.tile([128, QPAD], f32)
            s8_sb = pp.tile([128, N_GRP * 32], f32)
            nc.sync.dma_start(out=idx_sb[:], in_=idxw[:])
            nc.sync.dma_start(out=xqc_sb[:], in_=xqcw[:])
            nc.sync.dma_start(out=pwc_sb[:], in_=pwcw[:])
            nc.sync.dma_start(out=wkc_s[:], in_=WkCm[:])
            nc.sync.dma_start(out=wv_s[:], in_=Wvm[:])
            nc.sync.dma_start(out=wp2c_s[:], in_=Wp2Cm[:])
            nc.sync.dma_start(out=wp2_s[:], in_=Wp2m[:])
            nc.sync.dma_start(out=l6_s[:], in_=L6m[:])
            nc.sync.dma_start(out=m3_s[:], in_=M3m[:])
            nc.sync.dma_start(out=m1_s[:], in_=M1m[:])
            nc.sync.dma_start(out=w1c_s[:], in_=W1cm[:])
            nc.sync.dma_start(out=wm32_s[:], in_=Wm32m[:])
            nc.sync.dma_start(out=o8_s[:], in_=O8m[:])
            nc.sync.dma_start(out=cv[:], in_=cvec[:])

            # Pre-load the one ACT table that covers every activation we
            # use (ln, exp, square, relu, copy). Without this the
            # insert_act_table_loads pass greedily alternates natural_log
            # and exp_and_others, reloading tables (1.3us) twice per chunk.
            from concourse.hw_specs import get_activation_tables
            _tables = list(get_activation_tables(nc.m.arch).keys())
            _set_id = _tables.index("natural_log_exp_and_others")
            _ld = mybir.InstLoadActFuncSet(
                name=nc.get_next_instruction_name(), ins=[], outs=[],
                act_func_set_id=_set_id)
            nc.scalar.add_instruction(_ld)

            with tc.tile_pool(name="og", bufs=5) as pog, \
                 tc.tile_pool(name="wk", bufs=3) as pw, \
                 tc.tile_pool(name="vp", bufs=6) as pv, \
                 tc.tile_pool(name="e8", bufs=2) as pe8, \
                 tc.tile_pool(name="pkv", bufs=2, space="PSUM") as ppkv, \
                 tc.tile_pool(name="ppe", bufs=2, space="PSUM") as pppe, \
                 tc.tile_pool(name="pmt", bufs=1, space="PSUM") as ppmt, \
                 tc.tile_pool(name="pw1", bufs=2, space="PSUM") as ppw1, \
                 tc.tile_pool(name="pwm", bufs=1, space="PSUM") as ppwm:
                for grp in range(N_GRP):
                    if grp % GPU_ == 0:
                        og = pog.tile([128, 2, E_GATHER], f16, tag="og")
                        gu = grp // GPU_
                        nc.gpsimd.dma_gather(
                            og[:], gtab.ap(),
                            idx_sb[:, gu * (E_GATHER // 16):
                                   (gu + 1) * (E_GATHER // 16)],
                            num_idxs=E_GATHER, num_idxs_reg=E_GATHER,
                            elem_size=TROW, transpose=True,
                            single_packet=False)
                    wmb = ppwm.tile([128, E_CHUNK], f32, tag="wmb")
                    vpes = []
                    for j in range(GRP):
                        ci = grp * GRP + j
                        pt0 = ci * CHUNK
                        pt1 = pt0 + CHUNK
                        sl = slice(j * E_CHUNK, (j + 1) * E_CHUNK)
                        ogx = og[:, 0, sl]
                        ogp = og[0:6, 1, sl]
                        # ---- pe chain (3 channels, centered) ----
                        # cc3p and ms3 share one PSUM bank (rows 0:3 / 32:35)
                        pe_t = pppe.tile([64, E_CHUNK], f32, tag="pe")
                        nc.tensor.matmul(pe_t[0:3, :], lhsT=l6_s[0:6, 0:3],
                                         rhs=ogp, start=True, stop=True)
                        cc3 = pw.tile([4, E_CHUNK], f16, tag="cc3")
                        nc.vector.scalar_tensor_tensor(
                            cc3[0:3].rearrange("p (n s) -> p n s", s=S),
                            pe_t[0:3].rearrange("p (n s) -> p n s", s=S),
                            0.0,
                            pwc_sb[0:3, pt0:pt1].to_broadcast([3, CHUNK, S]),
                            op0=ALU.add, op1=ALU.subtract)
                        sq3 = pw.tile([4, E_CHUNK], f16, tag="sq3")
                        nc.scalar.activation(sq3[0:3], cc3[0:3], AF.Square)
                        nc.tensor.matmul(pe_t[32:35, :], lhsT=m3_s[0:3, 0:3],
                                         rhs=sq3[0:3], start=True, stop=True)
                        l3 = pw.tile([4, E_CHUNK], f16, tag="l3")
                        nc.scalar.activation(l3[0:3], pe_t[32:35], AF.Ln,
                                             bias=cv[0:3, 0:1])
                        iv3 = pw.tile([4, E_CHUNK], f16, tag="iv3")
                        nc.scalar.activation(iv3[0:3], l3[0:3], AF.Exp,
                                             scale=cv[0:3, 2:3])
                        a3 = pw.tile([4, E_CHUNK], f16, tag="a3")
                        nc.vector.scalar_tensor_tensor(
                            a3[0:3], cc3[0:3], 0.0, iv3[0:3],
                            op0=ALU.max, op1=ALU.mult)
                        # ---- centered k bank; cr = bank - xqC ----
                        kc = ppkv.tile([128, E_CHUNK], f32, tag="kv")
                        nc.tensor.matmul(kc[:], lhsT=wkc_s[:], rhs=ogx,
                                         start=True, stop=False)
                        nc.tensor.matmul(kc[:], lhsT=wp2c_s[0:3, :],
                                         rhs=a3[0:3], start=False, stop=True)
                        cr = pw.tile([128, E_CHUNK], f16, tag="cr")
                        nc.vector.scalar_tensor_tensor(
                            cr[:].rearrange("p (n s) -> p n s", s=S),
                            kc[:].rearrange("p (n s) -> p n s", s=S),
                            0.0,
                            xqc_sb[:, pt0:pt1].to_broadcast([128, CHUNK, S]),
                            op0=ALU.add, op1=ALU.subtract)
                        rc1 = pw.tile([128, E_CHUNK], f16, tag="rc1")
                        nc.scalar.activation(rc1[:], cr[:], AF.Relu)
                        # ---- folded LN2 ----
                        w1 = ppw1.tile([128, E_CHUNK], f32, tag="w1")
                        nc.tensor.matmul(w1[:], lhsT=w1c_s[:], rhs=rc1[:],
                                         start=True, stop=True)
                        sqw = pw.tile([128, E_CHUNK], f16, tag="sqw")
                        nc.scalar.activation(sqw[:], w1[:], AF.Square)
                        mt = ppmt.tile([128, E_CHUNK], f32, tag="mt")
                        nc.tensor.matmul(mt[:], lhsT=m1_s[:], rhs=sqw[:],
                                         start=True, stop=True)
                        lw = pw.tile([128, E_CHUNK], f16, tag="lw")
                        nc.scalar.activation(lw[:], mt[:], AF.Ln,
                                             bias=cv[:, 1:2])
                        ivw = pw.tile([128, E_CHUNK], f16, tag="ivw")
                        nc.scalar.activation(ivw[:], lw[:], AF.Exp,
                                             scale=cv[:, 2:3])
                        y2 = pw.tile([128, E_CHUNK], f16, tag="y2")
                        nc.vector.scalar_tensor_tensor(
                            y2[:], w1[:], 0.0, ivw[:],
                            op0=ALU.max, op1=ALU.mult)
                        nc.tensor.matmul(wmb[32 * j:32 * j + 32, :],
                                         lhsT=wm32_s[:], rhs=y2[:],
                                         start=True, stop=True)
                        # ---- v + peT; stage out of PSUM ----
                        vb = ppkv.tile([128, E_CHUNK], f32, tag="kv")
                        nc.tensor.matmul(vb[:], lhsT=wv_s[:], rhs=ogx,
                                         start=True, stop=False)
                        nc.tensor.matmul(vb[:], lhsT=wp2_s[0:3, :],
                                         rhs=a3[0:3], start=False, stop=True)
                        vpe = pv.tile([128, E_CHUNK], f16, tag="vpe")
                        nc.vector.tensor_copy(vpe[:], vb[:])
                        vpes.append(vpe)
                    # ---- batched softmax tail ----
                    e8 = pe8.tile([128, E_CHUNK], bf16, tag="e8")
                    nc.scalar.activation(e8[0:96], wmb[0:96], AF.Exp)
                    nc.vector.tensor_reduce(
                        s8_sb[0:96, grp * 32:(grp + 1) * 32],
                        e8[0:96].rearrange("p (n s) -> p n s", s=S),
                        axis=AX.X, op=ALU.add)
                    for j in range(GRP):
                        ci = grp * GRP + j
                        wbig = ppw1.tile([128, E_CHUNK], f32, tag="w1")
                        nc.tensor.matmul(wbig[:],
                                         lhsT=o8_s[32 * j:32 * j + 8, :],
                                         rhs=e8[32 * j:32 * j + 8, :],
                                         start=True, stop=True)
                        prod = pw.tile([128, E_CHUNK], f32, tag="prod")
                        nc.vector.scalar_tensor_tensor(
                            prod[:], wbig[:], 0.0, vpes[j][:],
                            op0=ALU.add, op1=ALU.mult)
                        nc.vector.tensor_reduce(
                            out_sb[:, ci * 32:(ci + 1) * 32],
                            prod[:].rearrange("p (n s) -> p n s", s=S),
                            axis=AX.X, op=ALU.add)
            nc.sync.dma_start(out=outT[:], in_=out_sb[:])
            nc.sync.dma_start(out=s8T[:], in_=s8_sb[:])
    nc.finalize()
    return nc


def cvec_const():
    cvec = np.zeros((128, 4), np.float32)
    cvec[:, 0] = EPS
    cvec[:, 1] = EPS * EPS
    cvec[:, 2] = -0.5
    return cvec


def _host_prep(p, x, idx, Wq, bq, Wk, bk, Wv, bv, Wp1, bp1, gp, betap, Wp2,
               bp2, gw1, betaw1, Ww1, bw1, gw2, betaw2, Ww2, bw2):
    f16 = np.float16
    import ml_dtypes

    # the device program is specialized to identity LN affines / zero bw1
    if not (np.allclose(gp, 1) and np.allclose(betap, 0)
            and np.allclose(gw1, 1) and np.allclose(betaw1, 0)
            and np.allclose(gw2, 1) and np.allclose(betaw2, 0)
            and np.allclose(bw1, 0)):
        raise RuntimeError("kernel specialized to gamma=1/beta=0 LN affines")

    Mc = np.eye(3, dtype=np.float32) - 1.0 / 3.0
    M1 = np.zeros((128, 128), np.float32)
    for h in range(H):
        M1[h * D:(h + 1) * D, h * D:(h + 1) * D] = 1.0 / D
    CM1 = np.eye(128, dtype=np.float32) - M1
    Bd = np.zeros((128, 128), np.float32)
    for h in range(H):
        Bd[h * D:(h + 1) * D, h * OS:(h + 1) * OS] = Ww1
    W1c = Bd @ CM1
    ww2m = (Ww2 @ np.ones((OS,), np.float32)) / OS
    Wm32 = np.zeros((128, 32), np.float32)
    for h in range(H):
        Wm32[h * D:(h + 1) * D, h] = ww2m
    O8 = np.zeros((96, 128), np.float32)
    for j in range(3):
        for h in range(H):
            O8[32 * j + h, h * D:(h + 1) * D] = 1.0
    # Round Wp1@Mc to fp16 FIRST and use the rounded matrix on both the
    # edge side (L6, device) and the center side (pwc, host): cc3 is a
    # difference of near-equal terms for close neighbors, so both sides
    # must use bit-identical weights or the cancellation amplifies the
    # rounding error ~60x.
    WpMc = (Wp1 @ Mc).astype(f16).astype(np.float32)
    L6 = np.zeros((8, 4), np.float32)
    L6[0:3, 0:3] = WpMc
    L6[3:6, 0:3] = WpMc
    M3 = np.zeros((4, 4), np.float32)
    M3[0:3, 0:3] = 1.0 / 3.0
    Wp2f = np.zeros((4, 128), np.float32)
    Wp2f[0:3, :] = Wp2
    Wp2C = Wp2f @ CM1

    shared = {
        "WkCm": (Wk @ CM1).astype(f16), "Wvm": Wv.astype(f16),
        "Wp2Cm": Wp2C.astype(f16), "Wp2m": Wp2f.astype(f16),
        "L6m": L6.astype(f16), "M3m": M3.astype(f16),
        "M1m": M1.astype(f16),
        "W1cm": W1c.astype(f16), "Wm32m": Wm32.astype(f16),
        "O8m": O8.astype(ml_dtypes.bfloat16),
        "cvec": cvec_const(),
    }

    gtabs = []
    for cl in range(2):
        cs, ce = cl * NC_CLOUD, (cl + 1) * NC_CLOUD
        g = np.zeros((NC_CLOUD, TROW), f16)
        g[:, 0:128] = x[cs:ce].astype(f16)
        phi = p[cs:ce].astype(f16)
        plo = (p[cs:ce] - phi.astype(np.float32)).astype(f16)
        g[:, 128:131] = phi
        g[:, 131:134] = plo
        gtabs.append(g)

    biasKQ = bk + bp2 - bq
    in_maps = []
    for c in range(N_CORES):
        cloud = c // (N_CORES // 2)
        cs = cloud * NC_CLOUD
        base = (c % (N_CORES // 2)) * NP_CORE
        q0, q1 = cs + base, cs + base + NP_CORE
        # cr = CM1(k + peT - xq') with xqC = CM1 @ xq' precomputed here
        xq = (x[q0:q1].astype(np.float32) @ Wq) - biasKQ
        xqc = xq @ CM1                    # CM1 symmetric
        xqcw = np.zeros((QPAD, 128), f16)
        xqcw[:NP_CORE] = xqc.astype(f16)
        # cc3 = Mc Wp1^T p_j - pwc' must equal Mc(Wp1^T(p_j - p_n) + bp1)
        # so pwc' = Mc Wp1^T p_n - Mc bp1
        pwc = p[q0:q1].astype(np.float32) @ WpMc - Mc @ bp1
        pwcw = np.zeros((4, QPAD), np.float32)
        pwcw[0:3, :NP_CORE] = pwc.T
        jl = (idx[q0:q1] - cs).astype(np.int64).reshape(-1)
        flat = np.zeros((N_CHUNKS * E_CHUNK,), np.int16)
        flat[:NP_CORE * S] = jl.astype(np.int16)
        nci = E_GATHER // 16
        ngu = N_GRP // GPU_
        iw = np.zeros((128, ngu * nci), np.int16)
        for g in range(ngu):
            blk = flat[g * E_GATHER:(g + 1) * E_GATHER].reshape(nci, 16).T
            iw[:, g * nci:(g + 1) * nci] = np.tile(blk, (8, 1))
        m = {"gtab": gtabs[cloud], "idxw": iw,
             "xqcw": np.ascontiguousarray(xqcw.T), "pwcw": pwcw}
        m.update(shared)
        in_maps.append(m)
    return in_maps


def _host_post(results, bv, bp2):
    """Normalize by the softmax sums and add the v-side bias."""
    biasV = (bv + bp2).astype(np.float32)
    out = np.empty((N, O), dtype=np.float32)
    qs = np.arange(NP_CORE)
    ci = qs // CHUNK
    col = (ci // GRP) * 32 + (qs % CHUNK)
    row0 = (ci % GRP) * 32                       # + head
    for c in range(N_CORES):
        o = results[c]["outT"]                   # [128, QPAD]
        s8 = results[c]["s8T"]                   # [128, N_GRP*32]
        den = s8[(row0[None, :] + np.arange(H)[:, None]), col[None, :]]
        denc = np.repeat(den, D, axis=0)         # [128, NP_CORE]
        res = o[:, :NP_CORE] / denc + biasV[:, None]
        out[c * NP_CORE:(c + 1) * NP_CORE] = res.T
    return out


_BASS_CACHE = {}


def kernel(p, x, idx, Wq, bq, Wk, bk, Wv, bv, Wp1, bp1, gp, betap, Wp2, bp2,
           gw1, betaw1, Ww1, bw1, gw2, betaw2, Ww2, bw2):
    args = dict(p=_f32(p), x=_f32(x), idx=np.asarray(idx),
                Wq=_f32(Wq), bq=_f32(bq), Wk=_f32(Wk), bk=_f32(bk),
                Wv=_f32(Wv), bv=_f32(bv), Wp1=_f32(Wp1), bp1=_f32(bp1),
                gp=_f32(gp), betap=_f32(betap), Wp2=_f32(Wp2), bp2=_f32(bp2),
                gw1=_f32(gw1), betaw1=_f32(betaw1), Ww1=_f32(Ww1),
                bw1=_f32(bw1), gw2=_f32(gw2), betaw2=_f32(betaw2),
                Ww2=_f32(Ww2), bw2=_f32(bw2))
    try:
        import sys
        if "/opt/trn_rl_repo" not in sys.path:
            sys.path.insert(0, "/opt/trn_rl_repo")
        from concourse.bass_utils import run_bass_kernel_spmd
        in_maps = _host_prep(**args)
        if "nc" not in _BASS_CACHE:
            _BASS_CACHE["nc"] = _build_bass()
        nc = _BASS_CACHE["nc"]
        res = run_bass_kernel_spmd(nc, in_maps, list(range(N_CORES)))
        return _host_post(res.results, args["bv"], args["bp2"])
    except Exception:
        import traceback
        traceback.print_exc()
        return _kernel_numpy(**args)


# revision 33
# speedup vs baseline: 1.1870x; 1.1870x over previous
import numpy as np

N, C, O, H, S = 30000, 128, 128, 8, 16
D = O // H            # 16
OS = 16
EPS = 1e-5
N_CORES = 8
OFFSETS = [15000, 30000]
NC_CLOUD = 15000          # points per cloud
NP_CORE = N // N_CORES    # 3750 query points per core
CHUNK = 32                # query points per compute chunk
E_CHUNK = CHUNK * S       # 512 edges per chunk
GRP = 3                   # chunks per softmax group (PSUM offsets 0/32/64)
N_GRP = 40                # groups per core (120 chunks, 3840 query slots)
N_CHUNKS = N_GRP * GRP
QPAD = N_CHUNKS * CHUNK   # 3840
E_GRP = GRP * E_CHUNK     # 2048 edges gathered per group
TROW = 256                # fp16 slots per gather row: x(128)|p_hi(3)|p_lo(3)|pad
GPU_ = 1                  # softmax groups per gather unit
E_GATHER = GPU_ * E_GRP   # idxs per dma_gather
SQ_SCALE = 2.0 ** -14     # sqr1 = SQ_SCALE*cr^2 (keeps eps*vr matmul weights normal)


def _f32(a):
    return np.ascontiguousarray(np.asarray(a, dtype=np.float32))


def _ln_np(x, g, b):
    m = x.mean(-1, keepdims=True, dtype=np.float32)
    v = ((x - m) ** 2).mean(-1, keepdims=True, dtype=np.float32)
    return (x - m) / np.sqrt(v + EPS) * g + b


def _kernel_numpy(p, x, idx, Wq, bq, Wk, bk, Wv, bv, Wp1, bp1, gp, betap, Wp2,
                  bp2, gw1, betaw1, Ww1, bw1, gw2, betaw2, Ww2, bw2):
    xq = (x @ Wq + bq).reshape(N, H, D)
    xk = x @ Wk + bk
    xv = x @ Wv + bv
    out = np.empty((N, O), dtype=np.float32)
    CH = N // N_CORES
    for s0 in range(0, N, CH):
        s1 = min(s0 + CH, N)
        ii = idx[s0:s1]
        kg = xk[ii].reshape(-1, S, H, D)
        vg = xv[ii].reshape(-1, S, H, D)
        pr = p[ii] - p[s0:s1, None, :]
        t = _ln_np(pr @ Wp1 + bp1, gp, betap)
        pe = np.maximum(t, 0.0) @ Wp2 + bp2
        pe = pe.reshape(-1, S, H, D)
        r = kg + pe - xq[s0:s1, None]
        w = np.maximum(_ln_np(r, gw1, betaw1), 0.0) @ Ww1 + bw1
        w = np.maximum(_ln_np(w, gw2, betaw2), 0.0) @ Ww2 + bw2
        wm = w.mean(-1, dtype=np.float32)
        wm = wm - wm.max(axis=1, keepdims=True)
        e = np.exp(wm)
        wsm = e / e.sum(axis=1, keepdims=True)
        out[s0:s1] = ((vg + pe) * wsm[..., None]).sum(axis=1).reshape(s1 - s0, O)
    return out


def _build_bass():
    """SPMD Bass program: one NeuronCore handles 3750 query points.

    Table-free: per group of 3 chunks one dma_gather of x|p rows
    (512 B/edge) straight from a host-built DRAM array. Specialized to
    gamma=1/beta=0 LayerNorm affines, which lets relu factor through the
    positive LN scales so LN1's rsqrt cancels out of LN2 exactly:
        cwh = w~ * rsqrt(m~ + eps^2)   (the eps*vr term is negligible).
    The remaining two rsqrts run as Ln + Exp(-0.5*) pairs so every
    activation (ln, exp, relu, square, copy) lives in one ACT table.
    Softmax exp is batched 3 chunks per PSUM bank (partition offsets
    0/32/64); normalization and output biases are applied on the host.
    """
    import concourse.bass as bass
    import concourse.bacc as bacc
    import concourse.tile as tile
    from concourse import mybir

    f32 = mybir.dt.float32
    f16 = mybir.dt.float16
    bf16 = mybir.dt.bfloat16
    i16 = mybir.dt.int16
    AF = mybir.ActivationFunctionType
    ALU = mybir.AluOpType
    AX = mybir.AxisListType

    nc = bacc.Bacc("TRN2", target_bir_lowering=False, debug=False,
                   num_devices=N_CORES)

    gtab = nc.declare_dram_parameter("gtab", [NC_CLOUD, TROW], f16,
                                     isOutput=False)
    idxw = nc.declare_dram_parameter("idxw", [128, (N_GRP // GPU_) * (E_GATHER // 16)], i16,
                                     isOutput=False)
    xqcw = nc.declare_dram_parameter("xqcw", [128, QPAD], f16, isOutput=False)
    pwcw = nc.declare_dram_parameter("pwcw", [4, QPAD], f32, isOutput=False)
    WkCm = nc.declare_dram_parameter("WkCm", [128, 128], f16, isOutput=False)
    Wvm = nc.declare_dram_parameter("Wvm", [128, 128], f16, isOutput=False)
    Wp2Cm = nc.declare_dram_parameter("Wp2Cm", [4, 128], f16, isOutput=False)
    Wp2m = nc.declare_dram_parameter("Wp2m", [4, 128], f16, isOutput=False)
    L6m = nc.declare_dram_parameter("L6m", [8, 4], f16, isOutput=False)
    M3m = nc.declare_dram_parameter("M3m", [4, 4], f16, isOutput=False)
    M1m = nc.declare_dram_parameter("M1m", [128, 128], f16, isOutput=False)
    W1cm = nc.declare_dram_parameter("W1cm", [128, 128], f16, isOutput=False)
    Wm32m = nc.declare_dram_parameter("Wm32m", [128, 32], f16, isOutput=False)
    O8m = nc.declare_dram_parameter("O8m", [96, 128], bf16, isOutput=False)
    # [128, 4] f32 constants: 0: EPS  1: EPS^2  2: -0.5
    cvec = nc.declare_dram_parameter("cvec", [128, 4], f32, isOutput=False)

    outT = nc.declare_dram_parameter("outT", [128, QPAD], f32, isOutput=True)
    s8T = nc.declare_dram_parameter("s8T", [128, N_GRP * 32], f32,
                                    isOutput=True)

    with tile.TileContext(nc) as tc, \
         nc.allow_low_precision(reason="fp16 intermediates are intentional"):
        with tc.tile_pool(name="persist", bufs=1) as pp:
            idx_sb = pp.tile([128, (N_GRP // GPU_) * (E_GATHER // 16)], i16)
            xqc_sb = pp.tile([128, QPAD], f16)
            pwc_sb = pp.tile([4, QPAD], f32)
            wkc_s = pp.tile([128, 128], f16)
            wv_s = pp.tile([128, 128], f16)
            wp2c_s = pp.tile([4, 128], f16)
            wp2_s = pp.tile([4, 128], f16)
            l6_s = pp.tile([8, 4], f16)
            m3_s = pp.tile([4, 4], f16)
            m1_s = pp.tile([128, 128], f16)
            w1c_s = pp.tile([128, 128], f16)
            wm32_s = pp.tile([128, 32], f16)
            o8_s = pp.tile([96, 128], bf16)
            cv = pp.tile([128, 4], f32)
            out_sb = pp.tile([128, QPAD], f32)
            s8_sb = pp.tile([128, N_GRP * 32], f32)
            nc.sync.dma_start(out=idx_sb[:], in_=idxw[:])
            nc.sync.dma_start(out=xqc_sb[:], in_=xqcw[:])
            nc.sync.dma_start(out=pwc_sb[:], in_=pwcw[:])
            nc.sync.dma_start(out=wkc_s[:], in_=WkCm[:])
            nc.sync.dma_start(out=wv_s[:], in_=Wvm[:])
            nc.sync.dma_start(out=wp2c_s[:], in_=Wp2Cm[:])
            nc.sync.dma_start(out=wp2_s[:], in_=Wp2m[:])
            nc.sync.dma_start(out=l6_s[:], in_=L6m[:])
            nc.sync.dma_start(out=m3_s[:], in_=M3m[:])
            nc.sync.dma_start(out=m1_s[:], in_=M1m[:])
            nc.sync.dma_start(out=w1c_s[:], in_=W1cm[:])
            nc.sync.dma_start(out=wm32_s[:], in_=Wm32m[:])
            nc.sync.dma_start(out=o8_s[:], in_=O8m[:])
            nc.sync.dma_start(out=cv[:], in_=cvec[:])

            # Pre-load the one ACT table that covers every activation we
            # use (ln, exp, square, relu, copy). Without this the
            # insert_act_table_loads pass greedily alternates natural_log
            # and exp_and_others, reloading tables (1.3us) twice per chunk.
            from concourse.hw_specs import get_activation_tables
            _tables = list(get_activation_tables(nc.m.arch).keys())
            _set_id = _tables.index("natural_log_exp_and_others")
            _ld = mybir.InstLoadActFuncSet(
                name=nc.get_next_instruction_name(), ins=[], outs=[],
                act_func_set_id=_set_id)
            nc.scalar.add_instruction(_ld)

            with tc.tile_pool(name="og", bufs=5) as pog, \
                 tc.tile_pool(name="wk", bufs=4) as pw, \
                 tc.tile_pool(name="vp", bufs=6) as pv, \
                 tc.tile_pool(name="e8", bufs=2) as pe8, \
                 tc.tile_pool(name="pkv", bufs=2, space="PSUM") as ppkv, \
                 tc.tile_pool(name="ppe", bufs=2, space="PSUM") as pppe, \
                 tc.tile_pool(name="pmt", bufs=1, space="PSUM") as ppmt, \
                 tc.tile_pool(name="pw1", bufs=2, space="PSUM") as ppw1, \
                 tc.tile_pool(name="pwm", bufs=1, space="PSUM") as ppwm:
                for grp in range(N_GRP):
                    if grp % GPU_ == 0:
                        og = pog.tile([128, 2, E_GATHER], f16, tag="og")
                        gu = grp // GPU_
                        nc.gpsimd.dma_gather(
                            og[:], gtab.ap(),
                            idx_sb[:, gu * (E_GATHER // 16):
                                   (gu + 1) * (E_GATHER // 16)],
                            num_idxs=E_GATHER, num_idxs_reg=E_GATHER,
                            elem_size=TROW, transpose=True,
                            single_packet=False)
                    wmb = ppwm.tile([128, E_CHUNK], f32, tag="wmb")
                    vpes = []
                    for j in range(GRP):
                        ci = grp * GRP + j
                        pt0 = ci * CHUNK
                        pt1 = pt0 + CHUNK
                        sl = slice(j * E_CHUNK, (j + 1) * E_CHUNK)
                        ogx = og[:, 0, sl]
                        ogp = og[0:6, 1, sl]
                        # ---- pe chain (3 channels, centered) ----
                        # cc3p and ms3 share one PSUM bank (rows 0:3 / 32:35)
                        pe_t = pppe.tile([64, E_CHUNK], f32, tag="pe")
                        nc.tensor.matmul(pe_t[0:3, :], lhsT=l6_s[0:6, 0:3],
                                         rhs=ogp, start=True, stop=True)
                        cc3 = pw.tile([4, E_CHUNK], f16, tag="cc3")
                        nc.vector.scalar_tensor_tensor(
                            cc3[0:3].rearrange("p (n s) -> p n s", s=S),
                            pe_t[0:3].rearrange("p (n s) -> p n s", s=S),
                            0.0,
                            pwc_sb[0:3, pt0:pt1].to_broadcast([3, CHUNK, S]),
                            op0=ALU.add, op1=ALU.subtract)
                        sq3 = pw.tile([4, E_CHUNK], f16, tag="sq3")
                        nc.scalar.activation(sq3[0:3], cc3[0:3], AF.Square)
                        nc.tensor.matmul(pe_t[32:35, :], lhsT=m3_s[0:3, 0:3],
                                         rhs=sq3[0:3], start=True, stop=True)
                        l3 = pw.tile([4, E_CHUNK], f16, tag="l3")
                        nc.scalar.activation(l3[0:3], pe_t[32:35], AF.Ln,
                                             bias=cv[0:3, 0:1])
                        iv3 = pw.tile([4, E_CHUNK], f16, tag="iv3")
                        nc.scalar.activation(iv3[0:3], l3[0:3], AF.Exp,
                                             scale=cv[0:3, 2:3])
                        a3 = pw.tile([4, E_CHUNK], f16, tag="a3")
                        nc.vector.scalar_tensor_tensor(
                            a3[0:3], cc3[0:3], 0.0, iv3[0:3],
                            op0=ALU.max, op1=ALU.mult)
                        # ---- centered k bank; cr = bank - xqC ----
                        kc = ppkv.tile([128, E_CHUNK], f32, tag="kv")
                        nc.tensor.matmul(kc[:], lhsT=wkc_s[:], rhs=ogx,
                                         start=True, stop=False)
                        nc.tensor.matmul(kc[:], lhsT=wp2c_s[0:3, :],
                                         rhs=a3[0:3], start=False, stop=True)
                        cr = pw.tile([128, E_CHUNK], f16, tag="cr")
                        nc.vector.scalar_tensor_tensor(
                            cr[:].rearrange("p (n s) -> p n s", s=S),
                            kc[:].rearrange("p (n s) -> p n s", s=S),
                            0.0,
                            xqc_sb[:, pt0:pt1].to_broadcast([128, CHUNK, S]),
                            op0=ALU.add, op1=ALU.subtract)
                        rc1 = pw.tile([128, E_CHUNK], f16, tag="rc1")
                        nc.scalar.activation(rc1[:], cr[:], AF.Relu)
                        # ---- folded LN2 ----
                        w1 = ppw1.tile([128, E_CHUNK], f32, tag="w1")
                        nc.tensor.matmul(w1[:], lhsT=w1c_s[:], rhs=rc1[:],
                                         start=True, stop=True)
                        sqw = pw.tile([128, E_CHUNK], f16, tag="sqw")
                        nc.scalar.activation(sqw[:], w1[:], AF.Square)
                        mt = ppmt.tile([128, E_CHUNK], f32, tag="mt")
                        nc.tensor.matmul(mt[:], lhsT=m1_s[:], rhs=sqw[:],
                                         start=True, stop=True)
                        lw = pw.tile([128, E_CHUNK], f16, tag="lw")
                        nc.scalar.activation(lw[:], mt[:], AF.Ln,
                                             bias=cv[:, 1:2])
                        ivw = pw.tile([128, E_CHUNK], f16, tag="ivw")
                        nc.scalar.activation(ivw[:], lw[:], AF.Exp,
                                             scale=cv[:, 2:3])
                        y2 = pw.tile([128, E_CHUNK], f16, tag="y2")
                        nc.vector.scalar_tensor_tensor(
                            y2[:], w1[:], 0.0, ivw[:],
                            op0=ALU.max, op1=ALU.mult)
                        nc.tensor.matmul(wmb[32 * j:32 * j + 32, :],
                                         lhsT=wm32_s[:], rhs=y2[:],
                                         start=True, stop=True)
                        # ---- v + peT; stage out of PSUM ----
                        vb = ppkv.tile([128, E_CHUNK], f32, tag="kv")
                        nc.tensor.matmul(vb[:], lhsT=wv_s[:], rhs=ogx,
                                         start=True, stop=False)
                        nc.tensor.matmul(vb[:], lhsT=wp2_s[0:3, :],
                                         rhs=a3[0:3], start=False, stop=True)
                        vpe = pv.tile([128, E_CHUNK], f16, tag="vpe")
                        nc.vector.tensor_copy(vpe[:], vb[:])
                        vpes.append(vpe)
                    # ---- batched softmax tail ----
                    e8 = pe8.tile([128, E_CHUNK], bf16, tag="e8")
                    nc.scalar.activation(e8[0:96], wmb[0:96], AF.Exp)
                    nc.vector.tensor_reduce(
                        s8_sb[0:96, grp * 32:(grp + 1) * 32],
                        e8[0:96].rearrange("p (n s) -> p n s", s=S),
                        axis=AX.X, op=ALU.add)
                    for j in range(GRP):
                        ci = grp * GRP + j
                        wbig = ppw1.tile([128, E_CHUNK], f32, tag="w1")
                        nc.tensor.matmul(wbig[:],
                                         lhsT=o8_s[32 * j:32 * j + 8, :],
                                         rhs=e8[32 * j:32 * j + 8, :],
                                         start=True, stop=True)
                        prod = pw.tile([128, E_CHUNK], f32, tag="prod")
                        nc.vector.scalar_tensor_tensor(
                            prod[:], wbig[:], 0.0, vpes[j][:],
                            op0=ALU.add, op1=ALU.mult)
                        nc.vector.tensor_reduce(
                            out_sb[:, ci * 32:(ci + 1) * 32],
                            prod[:].rearrange("p (n s) -> p n s", s=S),
                            axis=AX.X, op=ALU.add)
            nc.sync.dma_start(out=outT[:], in_=out_sb[:])
            nc.sync.dma_start(out=s8T[:], in_=s8_sb[:])
    nc.finalize()
    return nc


def cvec_const():
    cvec = np.zeros((128, 4), np.float32)
    cvec[:, 0] = EPS
    cvec[:, 1] = EPS * EPS
    cvec[:, 2] = -0.5
    return cvec


def _host_prep(p, x, idx, Wq, bq, Wk, bk, Wv, bv, Wp1, bp1, gp, betap, Wp2,
               bp2, gw1, betaw1, Ww1, bw1, gw2, betaw2, Ww2, bw2):
    f16 = np.float16
    import ml_dtypes

    # the device program is specialized to identity LN affines / zero bw1
    if not (np.allclose(gp, 1) and np.allclose(betap, 0)
            and np.allclose(gw1, 1) and np.allclose(betaw1, 0)
            and np.allclose(gw2, 1) and np.allclose(betaw2, 0)
            and np.allclose(bw1, 0)):
        raise RuntimeError("kernel specialized to gamma=1/beta=0 LN affines")

    Mc = np.eye(3, dtype=np.float32) - 1.0 / 3.0
    M1 = np.zeros((128, 128), np.float32)
    for h in range(H):
        M1[h * D:(h + 1) * D, h * D:(h + 1) * D] = 1.0 / D
    CM1 = np.eye(128, dtype=np.float32) - M1
    Bd = np.zeros((128, 128), np.float32)
    for h in range(H):
        Bd[h * D:(h + 1) * D, h * OS:(h + 1) * OS] = Ww1
    W1c = Bd @ CM1
    ww2m = (Ww2 @ np.ones((OS,), np.float32)) / OS
    Wm32 = np.zeros((128, 32), np.float32)
    for h in range(H):
        Wm32[h * D:(h + 1) * D, h] = ww2m
    O8 = np.zeros((96, 128), np.float32)
    for j in range(3):
        for h in range(H):
            O8[32 * j + h, h * D:(h + 1) * D] = 1.0
    # Round Wp1@Mc to fp16 FIRST and use the rounded matrix on both the
    # edge side (L6, device) and the center side (pwc, host): cc3 is a
    # difference of near-equal terms for close neighbors, so both sides
    # must use bit-identical weights or the cancellation amplifies the
    # rounding error ~60x.
    WpMc = (Wp1 @ Mc).astype(f16).astype(np.float32)
    L6 = np.zeros((8, 4), np.float32)
    L6[0:3, 0:3] = WpMc
    L6[3:6, 0:3] = WpMc
    M3 = np.zeros((4, 4), np.float32)
    M3[0:3, 0:3] = 1.0 / 3.0
    Wp2f = np.zeros((4, 128), np.float32)
    Wp2f[0:3, :] = Wp2
    Wp2C = Wp2f @ CM1

    shared = {
        "WkCm": (Wk @ CM1).astype(f16), "Wvm": Wv.astype(f16),
        "Wp2Cm": Wp2C.astype(f16), "Wp2m": Wp2f.astype(f16),
        "L6m": L6.astype(f16), "M3m": M3.astype(f16),
        "M1m": M1.astype(f16),
        "W1cm": W1c.astype(f16), "Wm32m": Wm32.astype(f16),
        "O8m": O8.astype(ml_dtypes.bfloat16),
        "cvec": cvec_const(),
    }

    gtabs = []
    for cl in range(2):
        cs, ce = cl * NC_CLOUD, (cl + 1) * NC_CLOUD
        g = np.zeros((NC_CLOUD, TROW), f16)
        g[:, 0:128] = x[cs:ce].astype(f16)
        phi = p[cs:ce].astype(f16)
        plo = (p[cs:ce] - phi.astype(np.float32)).astype(f16)
        g[:, 128:131] = phi
        g[:, 131:134] = plo
        gtabs.append(g)

    biasKQ = bk + bp2 - bq
    in_maps = []
    for c in range(N_CORES):
        cloud = c // (N_CORES // 2)
        cs = cloud * NC_CLOUD
        base = (c % (N_CORES // 2)) * NP_CORE
        q0, q1 = cs + base, cs + base + NP_CORE
        # cr = CM1(k + peT - xq') with xqC = CM1 @ xq' precomputed here
        xq = (x[q0:q1].astype(np.float32) @ Wq) - biasKQ
        xqc = xq @ CM1                    # CM1 symmetric
        xqcw = np.zeros((QPAD, 128), f16)
        xqcw[:NP_CORE] = xqc.astype(f16)
        # cc3 = Mc Wp1^T p_j - pwc' must equal Mc(Wp1^T(p_j - p_n) + bp1)
        # so pwc' = Mc Wp1^T p_n - Mc bp1
        pwc = p[q0:q1].astype(np.float32) @ WpMc - Mc @ bp1
        pwcw = np.zeros((4, QPAD), np.float32)
        pwcw[0:3, :NP_CORE] = pwc.T
        jl = (idx[q0:q1] - cs).astype(np.int64).reshape(-1)
        flat = np.zeros((N_CHUNKS * E_CHUNK,), np.int16)
        flat[:NP_CORE * S] = jl.astype(np.int16)
        nci = E_GATHER // 16
        ngu = N_GRP // GPU_
        iw = np.zeros((128, ngu * nci), np.int16)
        for g in range(ngu):
            blk = flat[g * E_GATHER:(g + 1) * E_GATHER].reshape(nci, 16).T
            iw[:, g * nci:(g + 1) * nci] = np.tile(blk, (8, 1))
        m = {"gtab": gtabs[cloud], "idxw": iw,
             "xqcw": np.ascontiguousarray(xqcw.T), "pwcw": pwcw}
        m.update(shared)
        in_maps.append(m)
    return in_maps


def _host_post(results, bv, bp2):
    """Normalize by the softmax sums and add the v-side bias."""
    biasV = (bv + bp2).astype(np.float32)
    out = np.empty((N, O), dtype=np.float32)
    qs = np.arange(NP_CORE)
    ci = qs // CHUNK
    col = (ci // GRP) * 32 + (qs % CHUNK)
    row0 = (ci % GRP) * 32                       # + head
    for c in range(N_CORES):
        o = results[c]["outT"]                   # [128, QPAD]
        s8 = results[c]["s8T"]                   # [128, N_GRP*32]
        den = s8[(row0[None, :] + np.arange(H)[:, None]), col[None, :]]
        denc = np.repeat(den, D, axis=0)         # [128, NP_CORE]
        res = o[:, :NP_CORE] / denc + biasV[:, None]
        out[c * NP_CORE:(c + 1) * NP_CORE] = res.T
    return out


_BASS_CACHE = {}


def kernel(p, x, idx, Wq, bq, Wk, bk, Wv, bv, Wp1, bp1, gp, betap, Wp2, bp2,
           gw1, betaw1, Ww1, bw1, gw2, betaw2, Ww2, bw2):
    args = dict(p=_f32(p), x=_f32(x), idx=np.asarray(idx),
                Wq=_f32(Wq), bq=_f32(bq), Wk=_f32(Wk), bk=_f32(bk),
                Wv=_f32(Wv), bv=_f32(bv), Wp1=_f32(Wp1), bp1=_f32(bp1),
                gp=_f32(gp), betap=_f32(betap), Wp2=_f32(Wp2), bp2=_f32(bp2),
                gw1=_f32(gw1), betaw1=_f32(betaw1), Ww1=_f32(Ww1),
                bw1=_f32(bw1), gw2=_f32(gw2), betaw2=_f32(betaw2),
                Ww2=_f32(Ww2), bw2=_f32(bw2))
    try:
        import sys
        if "/opt/trn_rl_repo" not in sys.path:
            sys.path.insert(0, "/opt/trn_rl_repo")
        from concourse.bass_utils import run_bass_kernel_spmd
        in_maps = _host_prep(**args)
        if "nc" not in _BASS_CACHE:
            _BASS_CACHE["nc"] = _build_bass()
        nc = _BASS_CACHE["nc"]
        res = run_bass_kernel_spmd(nc, in_maps, list(range(N_CORES)))
        return _host_post(res.results, args["bv"], args["bp2"])
    except Exception:
        import traceback
        traceback.print_exc()
        return _kernel_numpy(**args)


# revision 34
# speedup vs baseline: 1.1874x; 1.0003x over previous
import numpy as np

N, C, O, H, S = 30000, 128, 128, 8, 16
D = O // H            # 16
OS = 16
EPS = 1e-5
N_CORES = 8
OFFSETS = [15000, 30000]
NC_CLOUD = 15000          # points per cloud
NP_CORE = N // N_CORES    # 3750 query points per core
CHUNK = 32                # query points per compute chunk
E_CHUNK = CHUNK * S       # 512 edges per chunk
GRP = 3                   # chunks per softmax group (PSUM offsets 0/32/64)
N_GRP = 40                # groups per core (120 chunks, 3840 query slots)
N_CHUNKS = N_GRP * GRP
QPAD = N_CHUNKS * CHUNK   # 3840
E_GRP = GRP * E_CHUNK     # 2048 edges gathered per group
TROW = 256                # fp16 slots per gather row: x(128)|p_hi(3)|p_lo(3)|pad
GPU_ = 1                  # softmax groups per gather unit
E_GATHER = GPU_ * E_GRP   # idxs per dma_gather
SQ_SCALE = 2.0 ** -14     # sqr1 = SQ_SCALE*cr^2 (keeps eps*vr matmul weights normal)


def _f32(a):
    return np.ascontiguousarray(np.asarray(a, dtype=np.float32))


def _ln_np(x, g, b):
    m = x.mean(-1, keepdims=True, dtype=np.float32)
    v = ((x - m) ** 2).mean(-1, keepdims=True, dtype=np.float32)
    return (x - m) / np.sqrt(v + EPS) * g + b


def _kernel_numpy(p, x, idx, Wq, bq, Wk, bk, Wv, bv, Wp1, bp1, gp, betap, Wp2,
                  bp2, gw1, betaw1, Ww1, bw1, gw2, betaw2, Ww2, bw2):
    xq = (x @ Wq + bq).reshape(N, H, D)
    xk = x @ Wk + bk
    xv = x @ Wv + bv
    out = np.empty((N, O), dtype=np.float32)
    CH = N // N_CORES
    for s0 in range(0, N, CH):
        s1 = min(s0 + CH, N)
        ii = idx[s0:s1]
        kg = xk[ii].reshape(-1, S, H, D)
        vg = xv[ii].reshape(-1, S, H, D)
        pr = p[ii] - p[s0:s1, None, :]
        t = _ln_np(pr @ Wp1 + bp1, gp, betap)
        pe = np.maximum(t, 0.0) @ Wp2 + bp2
        pe = pe.reshape(-1, S, H, D)
        r = kg + pe - xq[s0:s1, None]
        w = np.maximum(_ln_np(r, gw1, betaw1), 0.0) @ Ww1 + bw1
        w = np.maximum(_ln_np(w, gw2, betaw2), 0.0) @ Ww2 + bw2
        wm = w.mean(-1, dtype=np.float32)
        wm = wm - wm.max(axis=1, keepdims=True)
        e = np.exp(wm)
        wsm = e / e.sum(axis=1, keepdims=True)
        out[s0:s1] = ((vg + pe) * wsm[..., None]).sum(axis=1).reshape(s1 - s0, O)
    return out


def _build_bass():
    """SPMD Bass program: one NeuronCore handles 3750 query points.

    Table-free: per group of 3 chunks one dma_gather of x|p rows
    (512 B/edge) straight from a host-built DRAM array. Specialized to
    gamma=1/beta=0 LayerNorm affines, which lets relu factor through the
    positive LN scales so LN1's rsqrt cancels out of LN2 exactly:
        cwh = w~ * rsqrt(m~ + eps^2)   (the eps*vr term is negligible).
    The remaining two rsqrts run as Ln + Exp(-0.5*) pairs so every
    activation (ln, exp, relu, square, copy) lives in one ACT table.
    Softmax exp is batched 3 chunks per PSUM bank (partition offsets
    0/32/64); normalization and output biases are applied on the host.
    """
    import concourse.bass as bass
    import concourse.bacc as bacc
    import concourse.tile as tile
    from concourse import mybir

    f32 = mybir.dt.float32
    f16 = mybir.dt.float16
    bf16 = mybir.dt.bfloat16
    i16 = mybir.dt.int16
    AF = mybir.ActivationFunctionType
    ALU = mybir.AluOpType
    AX = mybir.AxisListType

    nc = bacc.Bacc("TRN2", target_bir_lowering=False, debug=False,
                   num_devices=N_CORES)

    gtab = nc.declare_dram_parameter("gtab", [NC_CLOUD, TROW], f16,
                                     isOutput=False)
    idxw = nc.declare_dram_parameter("idxw", [128, (N_GRP // GPU_) * (E_GATHER // 16)], i16,
                                     isOutput=False)
    xqcw = nc.declare_dram_parameter("xqcw", [128, QPAD], f16, isOutput=False)
    pwcw = nc.declare_dram_parameter("pwcw", [4, QPAD], f32, isOutput=False)
    WkCm = nc.declare_dram_parameter("WkCm", [128, 128], f16, isOutput=False)
    Wvm = nc.declare_dram_parameter("Wvm", [128, 128], f16, isOutput=False)
    Wp2Cm = nc.declare_dram_parameter("Wp2Cm", [4, 128], f16, isOutput=False)
    Wp2m = nc.declare_dram_parameter("Wp2m", [4, 128], f16, isOutput=False)
    L6m = nc.declare_dram_parameter("L6m", [8, 4], f16, isOutput=False)
    M3m = nc.declare_dram_parameter("M3m", [4, 4], f16, isOutput=False)
    M1m = nc.declare_dram_parameter("M1m", [128, 128], f16, isOutput=False)
    W1cm = nc.declare_dram_parameter("W1cm", [128, 128], f16, isOutput=False)
    Wm32m = nc.declare_dram_parameter("Wm32m", [128, 32], f16, isOutput=False)
    O8m = nc.declare_dram_parameter("O8m", [96, 128], bf16, isOutput=False)
    # [128, 4] f32 constants: 0: EPS  1: EPS^2  2: -0.5
    cvec = nc.declare_dram_parameter("cvec", [128, 4], f32, isOutput=False)

    outT = nc.declare_dram_parameter("outT", [128, QPAD], f32, isOutput=True)
    s8T = nc.declare_dram_parameter("s8T", [128, N_GRP * 32], f32,
                                    isOutput=True)

    with tile.TileContext(nc) as tc, \
         nc.allow_low_precision(reason="fp16 intermediates are intentional"):
        with tc.tile_pool(name="persist", bufs=1) as pp:
            idx_sb = pp.tile([128, (N_GRP // GPU_) * (E_GATHER // 16)], i16)
            xqc_sb = pp.tile([128, QPAD], f16)
            pwc_sb = pp.tile([4, QPAD], f32)
            wkc_s = pp.tile([128, 128], f16)
            wv_s = pp.tile([128, 128], f16)
            wp2c_s = pp.tile([4, 128], f16)
            wp2_s = pp.tile([4, 128], f16)
            l6_s = pp.tile([8, 4], f16)
            m3_s = pp.tile([4, 4], f16)
            m1_s = pp.tile([128, 128], f16)
            w1c_s = pp.tile([128, 128], f16)
            wm32_s = pp.tile([128, 32], f16)
            o8_s = pp.tile([96, 128], bf16)
            cv = pp.tile([128, 4], f32)
            out_sb = pp.tile([128, QPAD], f32)
            s8_sb = pp.tile([128, N_GRP * 32], f32)
            nc.sync.dma_start(out=idx_sb[:], in_=idxw[:])
            nc.sync.dma_start(out=xqc_sb[:], in_=xqcw[:])
            nc.sync.dma_start(out=pwc_sb[:], in_=pwcw[:])
            nc.sync.dma_start(out=wkc_s[:], in_=WkCm[:])
            nc.sync.dma_start(out=wv_s[:], in_=Wvm[:])
            nc.sync.dma_start(out=wp2c_s[:], in_=Wp2Cm[:])
            nc.sync.dma_start(out=wp2_s[:], in_=Wp2m[:])
            nc.sync.dma_start(out=l6_s[:], in_=L6m[:])
            nc.sync.dma_start(out=m3_s[:], in_=M3m[:])
            nc.sync.dma_start(out=m1_s[:], in_=M1m[:])
            nc.sync.dma_start(out=w1c_s[:], in_=W1cm[:])
            nc.sync.dma_start(out=wm32_s[:], in_=Wm32m[:])
            nc.sync.dma_start(out=o8_s[:], in_=O8m[:])
            nc.sync.dma_start(out=cv[:], in_=cvec[:])

            # Pre-load the one ACT table that covers every activation we
            # use (ln, exp, square, relu, copy). Without this the
            # insert_act_table_loads pass greedily alternates natural_log
            # and exp_and_others, reloading tables (1.3us) twice per chunk.
            from concourse.hw_specs import get_activation_tables
            _tables = list(get_activation_tables(nc.m.arch).keys())
            _set_id = _tables.index("natural_log_exp_and_others")
            _ld = mybir.InstLoadActFuncSet(
                name=nc.get_next_instruction_name(), ins=[], outs=[],
                act_func_set_id=_set_id)
            nc.scalar.add_instruction(_ld)

            with tc.tile_pool(name="og", bufs=5) as pog, \
                 tc.tile_pool(name="wk", bufs=3) as pw, \
                 tc.tile_pool(name="vp", bufs=6) as pv, \
                 tc.tile_pool(name="e8", bufs=2) as pe8, \
                 tc.tile_pool(name="pkv", bufs=2, space="PSUM") as ppkv, \
                 tc.tile_pool(name="ppe", bufs=2, space="PSUM") as pppe, \
                 tc.tile_pool(name="pmt", bufs=1, space="PSUM") as ppmt, \
                 tc.tile_pool(name="pw1", bufs=2, space="PSUM") as ppw1, \
                 tc.tile_pool(name="pwm", bufs=1, space="PSUM") as ppwm:
                for grp in range(N_GRP):
                    if grp % GPU_ == 0:
                        og = pog.tile([128, 2, E_GATHER], f16, tag="og")
                        gu = grp // GPU_
                        nc.gpsimd.dma_gather(
                            og[:], gtab.ap(),
                            idx_sb[:, gu * (E_GATHER // 16):
                                   (gu + 1) * (E_GATHER // 16)],
                            num_idxs=E_GATHER, num_idxs_reg=E_GATHER,
                            elem_size=TROW, transpose=True,
                            single_packet=False)
                    wmb = ppwm.tile([128, E_CHUNK], f32, tag="wmb")
                    vpes = []
                    for j in range(GRP):
                        ci = grp * GRP + j
                        pt0 = ci * CHUNK
                        pt1 = pt0 + CHUNK
                        sl = slice(j * E_CHUNK, (j + 1) * E_CHUNK)
                        ogx = og[:, 0, sl]
                        ogp = og[0:6, 1, sl]
                        # ---- pe chain (3 channels, centered) ----
                        # cc3p and ms3 share one PSUM bank (rows 0:3 / 32:35)
                        pe_t = pppe.tile([64, E_CHUNK], f32, tag="pe")
                        nc.tensor.matmul(pe_t[0:3, :], lhsT=l6_s[0:6, 0:3],
                                         rhs=ogp, start=True, stop=True)
                        cc3 = pw.tile([4, E_CHUNK], f16, tag="cc3")
                        nc.vector.scalar_tensor_tensor(
                            cc3[0:3].rearrange("p (n s) -> p n s", s=S),
                            pe_t[0:3].rearrange("p (n s) -> p n s", s=S),
                            0.0,
                            pwc_sb[0:3, pt0:pt1].to_broadcast([3, CHUNK, S]),
                            op0=ALU.add, op1=ALU.subtract)
                        sq3 = pw.tile([4, E_CHUNK], f16, tag="sq3")
                        nc.scalar.activation(sq3[0:3], cc3[0:3], AF.Square)
                        nc.tensor.matmul(pe_t[32:35, :], lhsT=m3_s[0:3, 0:3],
                                         rhs=sq3[0:3], start=True, stop=True)
                        l3 = pw.tile([4, E_CHUNK], f16, tag="l3")
                        nc.scalar.activation(l3[0:3], pe_t[32:35], AF.Ln,
                                             bias=cv[0:3, 0:1])
                        iv3 = pw.tile([4, E_CHUNK], f16, tag="iv3")
                        nc.scalar.activation(iv3[0:3], l3[0:3], AF.Exp,
                                             scale=cv[0:3, 2:3])
                        a3 = pw.tile([4, E_CHUNK], f16, tag="a3")
                        nc.vector.scalar_tensor_tensor(
                            a3[0:3], cc3[0:3], 0.0, iv3[0:3],
                            op0=ALU.max, op1=ALU.mult)
                        # ---- centered k bank; cr = bank - xqC ----
                        kc = ppkv.tile([128, E_CHUNK], f32, tag="kv")
                        nc.tensor.matmul(kc[:], lhsT=wkc_s[:], rhs=ogx,
                                         start=True, stop=False)
                        nc.tensor.matmul(kc[:], lhsT=wp2c_s[0:3, :],
                                         rhs=a3[0:3], start=False, stop=True)
                        cr = pw.tile([128, E_CHUNK], f16, tag="cr")
                        nc.vector.scalar_tensor_tensor(
                            cr[:].rearrange("p (n s) -> p n s", s=S),
                            kc[:].rearrange("p (n s) -> p n s", s=S),
                            0.0,
                            xqc_sb[:, pt0:pt1].to_broadcast([128, CHUNK, S]),
                            op0=ALU.add, op1=ALU.subtract)
                        rc1 = pw.tile([128, E_CHUNK], f16, tag="rc1")
                        nc.scalar.activation(rc1[:], cr[:], AF.Relu)
                        # ---- folded LN2 ----
                        w1 = ppw1.tile([128, E_CHUNK], f32, tag="w1")
                        nc.tensor.matmul(w1[:], lhsT=w1c_s[:], rhs=rc1[:],
                                         start=True, stop=True)
                        sqw = pw.tile([128, E_CHUNK], f16, tag="sqw")
                        nc.scalar.activation(sqw[:], w1[:], AF.Square)
                        mt = ppmt.tile([128, E_CHUNK], f32, tag="mt")
                        nc.tensor.matmul(mt[:], lhsT=m1_s[:], rhs=sqw[:],
                                         start=True, stop=True)
                        lw = pw.tile([128, E_CHUNK], f16, tag="lw")
                        nc.scalar.activation(lw[:], mt[:], AF.Ln,
                                             bias=cv[:, 1:2])
                        ivw = pw.tile([128, E_CHUNK], f16, tag="ivw")
                        nc.scalar.activation(ivw[:], lw[:], AF.Exp,
                                             scale=cv[:, 2:3])
                        y2 = pw.tile([128, E_CHUNK], f16, tag="y2")
                        nc.vector.scalar_tensor_tensor(
                            y2[:], w1[:], 0.0, ivw[:],
                            op0=ALU.max, op1=ALU.mult)
                        nc.tensor.matmul(wmb[32 * j:32 * j + 32, :],
                                         lhsT=wm32_s[:], rhs=y2[:],
                                         start=True, stop=True)
                        # ---- v + peT; stage out of PSUM ----
                        vb = ppkv.tile([128, E_CHUNK], f32, tag="kv")
                        nc.tensor.matmul(vb[:], lhsT=wv_s[:], rhs=ogx,
                                         start=True, stop=False)
                        nc.tensor.matmul(vb[:], lhsT=wp2_s[0:3, :],
                                         rhs=a3[0:3], start=False, stop=True)
                        vpe = pv.tile([128, E_CHUNK], f16, tag="vpe")
                        nc.vector.tensor_copy(vpe[:], vb[:])
                        vpes.append(vpe)
                    # ---- batched softmax tail ----
                    e8 = pe8.tile([128, E_CHUNK], bf16, tag="e8")
                    nc.scalar.activation(e8[0:96], wmb[0:96], AF.Exp)
                    nc.vector.tensor_reduce(
                        s8_sb[0:96, grp * 32:(grp + 1) * 32],
                        e8[0:96].rearrange("p (n s) -> p n s", s=S),
                        axis=AX.X, op=ALU.add)
                    for j in range(GRP):
                        ci = grp * GRP + j
                        wbig = ppw1.tile([128, E_CHUNK], f32, tag="w1")
                        nc.tensor.matmul(wbig[:],
                                         lhsT=o8_s[32 * j:32 * j + 8, :],
                                         rhs=e8[32 * j:32 * j + 8, :],
                                         start=True, stop=True)
                        prod = pw.tile([128, E_CHUNK], f32, tag="prod")
                        nc.vector.scalar_tensor_tensor(
                            prod[:], wbig[:], 0.0, vpes[j][:],
                            op0=ALU.add, op1=ALU.mult)
                        nc.vector.tensor_reduce(
                            out_sb[:, ci * 32:(ci + 1) * 32],
                            prod[:].rearrange("p (n s) -> p n s", s=S),
                            axis=AX.X, op=ALU.add)
            nc.sync.dma_start(out=outT[:], in_=out_sb[:])
            nc.sync.dma_start(out=s8T[:], in_=s8_sb[:])
    nc.finalize()
    return nc


def cvec_const():
    cvec = np.zeros((128, 4), np.float32)
    cvec[:, 0] = EPS
    cvec[:, 1] = EPS * EPS
    cvec[:, 2] = -0.5
    return cvec


def _host_prep(p, x, idx, Wq, bq, Wk, bk, Wv, bv, Wp1, bp1, gp, betap, Wp2,
               bp2, gw1, betaw1, Ww1, bw1, gw2, betaw2, Ww2, bw2):
    f16 = np.float16
    import ml_dtypes

    # the device program is specialized to identity LN affines / zero bw1
    if not (np.allclose(gp, 1) and np.allclose(betap, 0)
            and np.allclose(gw1, 1) and np.allclose(betaw1, 0)
            and np.allclose(gw2, 1) and np.allclose(betaw2, 0)
            and np.allclose(bw1, 0)):
        raise RuntimeError("kernel specialized to gamma=1/beta=0 LN affines")

    Mc = np.eye(3, dtype=np.float32) - 1.0 / 3.0
    M1 = np.zeros((128, 128), np.float32)
    for h in range(H):
        M1[h * D:(h + 1) * D, h * D:(h + 1) * D] = 1.0 / D
    CM1 = np.eye(128, dtype=np.float32) - M1
    Bd = np.zeros((128, 128), np.float32)
    for h in range(H):
        Bd[h * D:(h + 1) * D, h * OS:(h + 1) * OS] = Ww1
    W1c = Bd @ CM1
    ww2m = (Ww2 @ np.ones((OS,), np.float32)) / OS
    Wm32 = np.zeros((128, 32), np.float32)
    for h in range(H):
        Wm32[h * D:(h + 1) * D, h] = ww2m
    O8 = np.zeros((96, 128), np.float32)
    for j in range(3):
        for h in range(H):
            O8[32 * j + h, h * D:(h + 1) * D] = 1.0
    # Round Wp1@Mc to fp16 FIRST and use the rounded matrix on both the
    # edge side (L6, device) and the center side (pwc, host): cc3 is a
    # difference of near-equal terms for close neighbors, so both sides
    # must use bit-identical weights or the cancellation amplifies the
    # rounding error ~60x.
    WpMc = (Wp1 @ Mc).astype(f16).astype(np.float32)
    L6 = np.zeros((8, 4), np.float32)
    L6[0:3, 0:3] = WpMc
    L6[3:6, 0:3] = WpMc
    M3 = np.zeros((4, 4), np.float32)
    M3[0:3, 0:3] = 1.0 / 3.0
    Wp2f = np.zeros((4, 128), np.float32)
    Wp2f[0:3, :] = Wp2
    Wp2C = Wp2f @ CM1

    shared = {
        "WkCm": (Wk @ CM1).astype(f16), "Wvm": Wv.astype(f16),
        "Wp2Cm": Wp2C.astype(f16), "Wp2m": Wp2f.astype(f16),
        "L6m": L6.astype(f16), "M3m": M3.astype(f16),
        "M1m": M1.astype(f16),
        "W1cm": W1c.astype(f16), "Wm32m": Wm32.astype(f16),
        "O8m": O8.astype(ml_dtypes.bfloat16),
        "cvec": cvec_const(),
    }

    gtabs = []
    for cl in range(2):
        cs, ce = cl * NC_CLOUD, (cl + 1) * NC_CLOUD
        g = np.zeros((NC_CLOUD, TROW), f16)
        g[:, 0:128] = x[cs:ce].astype(f16)
        phi = p[cs:ce].astype(f16)
        plo = (p[cs:ce] - phi.astype(np.float32)).astype(f16)
        g[:, 128:131] = phi
        g[:, 131:134] = plo
        gtabs.append(g)

    biasKQ = bk + bp2 - bq
    in_maps = []
    for c in range(N_CORES):
        cloud = c // (N_CORES // 2)
        cs = cloud * NC_CLOUD
        base = (c % (N_CORES // 2)) * NP_CORE
        q0, q1 = cs + base, cs + base + NP_CORE
        # cr = CM1(k + peT - xq') with xqC = CM1 @ xq' precomputed here
        xq = (x[q0:q1].astype(np.float32) @ Wq) - biasKQ
        xqc = xq @ CM1                    # CM1 symmetric
        xqcw = np.zeros((QPAD, 128), f16)
        xqcw[:NP_CORE] = xqc.astype(f16)
        # cc3 = Mc Wp1^T p_j - pwc' must equal Mc(Wp1^T(p_j - p_n) + bp1)
        # so pwc' = Mc Wp1^T p_n - Mc bp1
        pwc = p[q0:q1].astype(np.float32) @ WpMc - Mc @ bp1
        pwcw = np.zeros((4, QPAD), np.float32)
        pwcw[0:3, :NP_CORE] = pwc.T
        jl = (idx[q0:q1] - cs).astype(np.int64).reshape(-1)
        flat = np.zeros((N_CHUNKS * E_CHUNK,), np.int16)
        flat[:NP_CORE * S] = jl.astype(np.int16)
        nci = E_GATHER // 16
        ngu = N_GRP // GPU_
        iw = np.zeros((128, ngu * nci), np.int16)
        for g in range(ngu):
            blk = flat[g * E_GATHER:(g + 1) * E_GATHER].reshape(nci, 16).T
            iw[:, g * nci:(g + 1) * nci] = np.tile(blk, (8, 1))
        m = {"gtab": gtabs[cloud], "idxw": iw,
             "xqcw": np.ascontiguousarray(xqcw.T), "pwcw": pwcw}
        m.update(shared)
        in_maps.append(m)
    return in_maps


def _host_post(results, bv, bp2):
    """Normalize by the softmax sums and add the v-side bias."""
    biasV = (bv + bp2).astype(np.float32)
    out = np.empty((N, O), dtype=np.float32)
    qs = np.arange(NP_CORE)
    ci = qs // CHUNK
    col = (ci // GRP) * 32 + (qs % CHUNK)
    row0 = (ci % GRP) * 32                       # + head
    for c in range(N_CORES):
        o = results[c]["outT"]                   # [128, QPAD]
        s8 = results[c]["s8T"]                   # [128, N_GRP*32]
        den = s8[(row0[None, :] + np.arange(H)[:, None]), col[None, :]]
        denc = np.repeat(den, D, axis=0)         # [128, NP_CORE]
        res = o[:, :NP_CORE] / denc + biasV[:, None]
        out[c * NP_CORE:(c + 1) * NP_CORE] = res.T
    return out


_BASS_CACHE = {}


def kernel(p, x, idx, Wq, bq, Wk, bk, Wv, bv, Wp1, bp1, gp, betap, Wp2, bp2,
           gw1, betaw1, Ww1, bw1, gw2, betaw2, Ww2, bw2):
    args = dict(p=_f32(p), x=_f32(x), idx=np.asarray(idx),
                Wq=_f32(Wq), bq=_f32(bq), Wk=_f32(Wk), bk=_f32(bk),
                Wv=_f32(Wv), bv=_f32(bv), Wp1=_f32(Wp1), bp1=_f32(bp1),
                gp=_f32(gp), betap=_f32(betap), Wp2=_f32(Wp2), bp2=_f32(bp2),
                gw1=_f32(gw1), betaw1=_f32(betaw1), Ww1=_f32(Ww1),
                bw1=_f32(bw1), gw2=_f32(gw2), betaw2=_f32(betaw2),
                Ww2=_f32(Ww2), bw2=_f32(bw2))
    try:
        import sys
        if "/opt/trn_rl_repo" not in sys.path:
            sys.path.insert(0, "/opt/trn_rl_repo")
        from concourse.bass_utils import run_bass_kernel_spmd
        in_maps = _host_prep(**args)
        if "nc" not in _BASS_CACHE:
            _BASS_CACHE["nc"] = _build_bass()
        nc = _BASS_CACHE["nc"]
        res = run_bass_kernel_spmd(nc, in_maps, list(range(N_CORES)))
        return _host_post(res.results, args["bv"], args["bp2"])
    except Exception:
        import traceback
        traceback.print_exc()
        return _kernel_numpy(**args)
